# revision 6
# baseline (speedup 1.0000x reference)
"""Trainium2 Bass kernel for nn_Codebook (vq_codebook).

Reference computation (B=8192, IN_DIM=1024, K=2048, D=256, tau=1):
    h = x @ W + b
    h = batchnorm_train(h, gamma, beta)          # batch stats, biased var
    logits = -(|h|^2 - 2 h E^T + |e|^2)          # [B, K]
    g = -log(-log(u + 1e-10) + 1e-10)
    assignments = softmax(logits + g)            # tau = 1
    z = assignments @ E                          # [B, D]
    probs = softmax(logits)
    return probs, z

Sharding: data-parallel over B across 8 cores (1024 rows each); W/E and BN
scale/shift replicated, with a 2KB AllReduce for the batch mean/var.

Device algebra notes:
  - b cancels exactly in train-mode BN (shift invariance), so it is unused.
  - softmax is shift-invariant per row, so |h|^2 never needs computing:
    l = 2 h_n @ E^T - e2 gives the same softmax outputs.
  - The BN affine and the factor 2 fold into one per-feature scale/bias
    applied to h^T (feature dim on partitions).
  - -e2 enters the logits PSUM as one K=2 matmul of ones rows against the
    (hi, lo) split of -e2 (hi has 13 mantissa bits so fp32r keeps it exact).
  - Matmuls use bf16 hi/lo 3-pass decomposition (hi*hi + lo*hi + hi*lo),
    which measures ~1e-6 relative error: full PE rate, near-fp32 accuracy.
  - gumbel: with t = -log(u + 1e-10) + 1e-10 and lml = l - log(t),
    m2 = rowmax(lml):
        A_g = exp(lml - m2)                 (assignments numerator)
        A   = A_g * (-log u) = exp(l - m2)  (probs numerator; bounded by
              e^3.2 since l - m2 <= log t, so m2 stabilizes BOTH softmaxes)
    One Ln + one Ln + one Exp per element total (no reciprocal, no second
    Exp), and the rowmax reads SBUF instead of PSUM.
  - z^T = E^T @ A_g^T / S_a; the per-row 1/S_a is applied to z^T via a
    PE broadcast (ones x row of 1/S_a) and one fused multiply from PSUM.
"""

import os

import numpy as np

import concourse.bacc as bacc
import concourse.mybir as mybir
import concourse.tile as tile
from concourse.bass_utils import run_bass_kernel_spmd

N_CORES = 8
B, IN_DIM, K, D = 8192, 1024, 2048, 256
BC = B // N_CORES  # 1024 rows per core
BN_EPS = 1e-5

F32 = mybir.dt.float32
F32R = mybir.dt.float32r
BF16 = mybir.dt.bfloat16
AX = mybir.AxisListType
OP = mybir.AluOpType
AF = mybir.ActivationFunctionType

NKC = IN_DIM // 128  # 8   k-chunks of MM1
NDH = D // 128  # 2   feature halves
NBT = BC // 128  # 8   row tiles per core
NKQ = K // 512  # 4   logit column chunks
NKT = K // 128  # 16  code chunks (MM3 contraction)


def _build_program():
    nc = bacc.Bacc(num_devices=N_CORES, dynamic_dma_scratch_size=4096)

    xh_d = nc.dram_tensor("xh", [IN_DIM, BC], BF16, kind="ExternalInput")
    xl_d = nc.dram_tensor("xl", [IN_DIM, BC], BF16, kind="ExternalInput")
    u_d = nc.dram_tensor("u", [BC, K], F32, kind="ExternalInput")
    # W/E pre-laid-out on host: [128, chunk*D] with partition = k % 128
    Wh_d = nc.dram_tensor("Wh", [128, NKC * D], BF16, kind="ExternalInput")
    Wl_d = nc.dram_tensor("Wl", [128, NKC * D], BF16, kind="ExternalInput")
    gamma_d = nc.dram_tensor("gamma", [D, 1], F32, kind="ExternalInput")
    beta_d = nc.dram_tensor("beta", [D, 1], F32, kind="ExternalInput")
    El_d = nc.dram_tensor("El", [128, NKT * D], BF16, kind="ExternalInput")
    ETh_d = nc.dram_tensor("ETh", [D, K], BF16, kind="ExternalInput")
    ETl_d = nc.dram_tensor("ETl", [D, K], BF16, kind="ExternalInput")
    e2hl_d = nc.dram_tensor("e2hl", [2, K], F32R, kind="ExternalInput")
    eyeb_d = nc.dram_tensor("eyeb", [128, 128], BF16, kind="ExternalInput")
    eyer_d = nc.dram_tensor("eyer", [128, 128], F32R, kind="ExternalInput")
    probs_d = nc.dram_tensor("probs", [BC, K], F32, kind="ExternalOutput")
    zT_d = nc.dram_tensor("zT", [D, BC], F32, kind="ExternalOutput")

    with tile.TileContext(nc) as tc:
        _emit(nc, tc, xh_d, xl_d, u_d, Wh_d, Wl_d, gamma_d, beta_d, El_d,
              ETh_d, ETl_d, e2hl_d, eyeb_d, eyer_d, probs_d, zT_d)
    nc.finalize()
    return nc


def _emit(nc, tc, xh_d, xl_d, u_d, Wh_d, Wl_d, gamma_d, beta_d, El_d,
          ETh_d, ETl_d, e2hl_d, eyeb_d, eyer_d, probs_d, zT_d):
    from contextlib import ExitStack

    ctx = ExitStack()
    with ctx:
        const = ctx.enter_context(tc.tile_pool(name="const", bufs=1))
        hpool = ctx.enter_context(tc.tile_pool(name="hbuf", bufs=1))
        vec = ctx.enter_context(tc.tile_pool(name="vec", bufs=1))
        gum = ctx.enter_context(tc.tile_pool(name="gum", bufs=2))
        soft = ctx.enter_context(tc.tile_pool(name="soft", bufs=2))
        agt = ctx.enter_context(tc.tile_pool(name="agt", bufs=1))
        dram = ctx.enter_context(tc.tile_pool(name="dram", bufs=1, space="DRAM"))

        # ---- resident constants (emission order = DMA priority) -----------
        ETh_t = [const.tile([128, K], BF16, tag=f"ETh{dh}", name=f"ETh{dh}")
                 for dh in range(NDH)]
        ETl_t = [const.tile([128, K], BF16, tag=f"ETl{dh}", name=f"ETl{dh}")
                 for dh in range(NDH)]
        for dh in range(NDH):
            nc.sync.dma_start(ETh_t[dh][:], ETh_d[dh * 128:(dh + 1) * 128, :])
            nc.sync.dma_start(ETl_t[dh][:], ETl_d[dh * 128:(dh + 1) * 128, :])
        e2_t = const.tile([2, K], F32R, tag="e2")
        nc.sync.dma_start(e2_t[:], e2hl_d[:])
        eyeb_t = const.tile([128, 128], BF16, tag="eyeb")
        nc.sync.dma_start(eyeb_t[:], eyeb_d[:])
        eyer_t = const.tile([128, 128], F32R, tag="eyer")
        nc.sync.dma_start(eyer_t[:], eyer_d[:])
        gam_t = [const.tile([128, 1], F32, tag=f"gam{dh}", name=f"gam{dh}")
                 for dh in range(NDH)]
        bet_t = [const.tile([128, 1], F32, tag=f"bet{dh}", name=f"bet{dh}")
                 for dh in range(NDH)]
        for dh in range(NDH):
            nc.sync.dma_start(gam_t[dh][:], gamma_d[dh * 128:(dh + 1) * 128, :])
            nc.sync.dma_start(bet_t[dh][:], beta_d[dh * 128:(dh + 1) * 128, :])
        El_t = const.tile([128, NKT * D], BF16, tag="El")
        nc.sync.dma_start(El_t[:], El_d[:])
        ones2 = const.tile([2, 128], F32, tag="ones2")
        nc.gpsimd.memset(ones2[:], 1.0)
        ones_row = const.tile([1, 128], F32, tag="ones_row")
        nc.gpsimd.memset(ones_row[:], 1.0)
        lnbias = const.tile([128, 1], F32, tag="lnbias")
        nc.gpsimd.memset(lnbias[:], 1e-10)

        hh = [hpool.tile([128, BC], BF16, tag=f"hh{dh}", name=f"hh{dh}")
              for dh in range(NDH)]
        hl = [hpool.tile([128, BC], BF16, tag=f"hl{dh}", name=f"hl{dh}")
              for dh in range(NDH)]

        # ---- MM1 (3-pass bf16 hi/lo) + stats + AllReduce + BN -------------
        with (
            tc.tile_pool(name="mm1", bufs=1) as mm1p,
            tc.tile_pool(name="xload", bufs=3) as xpool,
            tc.tile_pool(name="ps_h", bufs=1, space="PSUM") as ps_h,
        ):
            Wh_t = mm1p.tile([128, NKC * D], BF16, tag="Wh")
            nc.sync.dma_start(Wh_t[:], Wh_d[:])
            Wl_t = mm1p.tile([128, NKC * D], BF16, tag="Wl")
            nc.sync.dma_start(Wl_t[:], Wl_d[:])
            ph = {}
            for dh in range(NDH):
                for bq in range(2):
                    ph[(dh, bq)] = ps_h.tile([128, 512], F32,
                                             tag=f"ph{dh}{bq}",
                                             name=f"ph{dh}{bq}")
            for kc in range(NKC):
                xh_t = xpool.tile([128, BC], BF16, tag="xh", name="xh")
                nc.sync.dma_start(xh_t[:], xh_d[kc * 128:(kc + 1) * 128, :])
                xl_t = xpool.tile([128, BC], BF16, tag="xl", name="xl")
                nc.sync.dma_start(xl_t[:], xl_d[kc * 128:(kc + 1) * 128, :])
                for dh in range(NDH):
                    wh_ap = Wh_t[:, kc * D + dh * 128: kc * D + dh * 128 + 128]
                    wl_ap = Wl_t[:, kc * D + dh * 128: kc * D + dh * 128 + 128]
                    # lhsT-grouped: Wh x {xh, xl}, then Wl x {xh}
                    for bq in range(2):
                        nc.tensor.matmul(
                            ph[(dh, bq)][:], wh_ap,
                            xh_t[:, bq * 512:(bq + 1) * 512],
                            start=(kc == 0), stop=False,
                        )
                    for bq in range(2):
                        nc.tensor.matmul(
                            ph[(dh, bq)][:], wh_ap,
                            xl_t[:, bq * 512:(bq + 1) * 512],
                            start=False, stop=False,
                        )
                    for bq in range(2):
                        nc.tensor.matmul(
                            ph[(dh, bq)][:], wl_ap,
                            xh_t[:, bq * 512:(bq + 1) * 512],
                            start=False, stop=(kc == NKC - 1),
                        )

            hsum_p = [vec.tile([128, 2], F32, tag=f"hsum{dh}",
                               name=f"hsum{dh}") for dh in range(NDH)]
            sqsum_p = [vec.tile([128, 2], F32, tag=f"sqsum{dh}",
                                name=f"sqsum{dh}") for dh in range(NDH)]
            for dh in range(NDH):
                sq_scr = mm1p.tile([128, 512], F32, tag="sqscr",
                                   name=f"sqscr{dh}")
                for bq in range(2):
                    p = ph[(dh, bq)]
                    nc.vector.tensor_reduce(
                        hsum_p[dh][:, bq:bq + 1], p[:], axis=AX.X, op=OP.add
                    )
                    nc.scalar.activation(
                        sq_scr[:], p[:], AF.Square,
                        accum_out=sqsum_p[dh][:, bq:bq + 1],
                    )

            stats_sb = vec.tile([128, 4], F32, tag="stats_sb")
            for dh in range(NDH):
                nc.vector.tensor_reduce(
                    stats_sb[:, dh:dh + 1], hsum_p[dh][:], axis=AX.X,
                    op=OP.add,
                )
                nc.vector.tensor_reduce(
                    stats_sb[:, 2 + dh:3 + dh], sqsum_p[dh][:], axis=AX.X,
                    op=OP.add,
                )
            cc_in = dram.tile([128, 4], F32)
            cc_out = dram.tile([128, 4], F32)
            nc.sync.dma_start(cc_in[:], stats_sb[:])
            nc.gpsimd.collective_compute(
                "AllReduce",
                OP.add,
                replica_groups=[list(range(N_CORES))],
                ins=[cc_in[:].opt()],
                outs=[cc_out[:].opt()],
            )
            stats_g = vec.tile([128, 4], F32, tag="stats_g")
            nc.sync.dma_start(stats_g[:], cc_out[:])

            # BN constants: scale2 = 2 s gamma, c2 = 2 (beta - mu s gamma)
            scale2 = []
            c2 = []
            for dh in range(NDH):
                mu = vec.tile([128, 1], F32, tag=f"mu{dh}", name=f"mu{dh}")
                nc.vector.tensor_scalar_mul(
                    mu[:], stats_g[:, dh:dh + 1], 1.0 / B
                )
                msq = vec.tile([128, 1], F32, tag=f"msq{dh}", name=f"msq{dh}")
                nc.vector.tensor_scalar_mul(
                    msq[:], stats_g[:, 2 + dh:3 + dh], 1.0 / B
                )
                negvar = vec.tile([128, 1], F32, tag=f"negvar{dh}",
                                  name=f"negvar{dh}")
                nc.vector.scalar_tensor_tensor(
                    negvar[:], mu[:], mu[:], msq[:],
                    op0=OP.mult, op1=OP.subtract,
                )
                veps = vec.tile([128, 1], F32, tag=f"veps{dh}",
                                name=f"veps{dh}")
                nc.vector.tensor_scalar(
                    veps[:], negvar[:], -1.0, BN_EPS, op0=OP.mult, op1=OP.add
                )
                rv = vec.tile([128, 1], F32, tag=f"rv{dh}", name=f"rv{dh}")
                nc.vector.reciprocal(rv[:], veps[:])
                s = vec.tile([128, 1], F32, tag=f"s{dh}", name=f"s{dh}")
                nc.scalar.activation(s[:], rv[:], AF.Sqrt)
                sg = vec.tile([128, 1], F32, tag=f"sg{dh}", name=f"sg{dh}")
                nc.vector.tensor_mul(sg[:], s[:], gam_t[dh][:])
                sc2 = vec.tile([128, 1], F32, tag=f"scale2{dh}",
                               name=f"scale2{dh}")
                nc.vector.tensor_add(sc2[:], sg[:], sg[:])
                musg = vec.tile([128, 1], F32, tag=f"musg{dh}",
                                name=f"musg{dh}")
                nc.vector.tensor_mul(musg[:], mu[:], sg[:])
                cb = vec.tile([128, 1], F32, tag=f"cb{dh}", name=f"cb{dh}")
                nc.vector.tensor_sub(cb[:], bet_t[dh][:], musg[:])
                cc2 = vec.tile([128, 1], F32, tag=f"c2{dh}", name=f"c2{dh}")
                nc.vector.tensor_add(cc2[:], cb[:], cb[:])
                scale2.append(sc2)
                c2.append(cc2)

            # h2n = 2 h_n from PSUM, then bf16 hi/lo split for MM2's lhsT
            for dh in range(NDH):
                h2n = mm1p.tile([128, BC], F32, tag="h2n", name=f"h2n{dh}")
                for bq in range(2):
                    nc.vector.tensor_scalar(
                        h2n[:, bq * 512:(bq + 1) * 512], ph[(dh, bq)][:],
                        scale2[dh][:], c2[dh][:], op0=OP.mult, op1=OP.add,
                    )
                nc.vector.tensor_copy(hh[dh][:], h2n[:])
                nc.vector.scalar_tensor_tensor(
                    hl[dh][:], h2n[:], 1.0, hh[dh][:],
                    op0=OP.bypass, op1=OP.subtract,
                )

        # ---- main loop over row tiles (pairs, to batch ACT table use) -----
        with (
            tc.tile_pool(name="ps_l", bufs=2, space="PSUM") as ps_l,
            tc.tile_pool(name="ps_scr", bufs=2, space="PSUM") as ps_scr,
            tc.tile_pool(name="ps_aux", bufs=1, space="PSUM") as ps_aux,
            tc.tile_pool(name="ps_z", bufs=1, space="PSUM") as ps_z,
        ):
            AgT = agt.tile([128, NKT * 512], BF16, tag="AgT")
            AgT4 = AgT[:].rearrange("p (a b c) -> p a b c", a=NKT, b=4, c=128)
            rs_row = [vec.tile([1, 512], F32R, tag=f"rsrow{g}",
                      name=f"rsrow{g}") for g in range(2)]

            lnus = {}
            lnts = {}

            def emit_gumbel(bt):
                ut = gum.tile([128, K], F32, tag="u", name=f"u{bt}")
                nc.sync.dma_start(ut[:], u_d[bt * 128:(bt + 1) * 128, :])
                lnu = gum.tile([128, K], F32, tag="lnu", name=f"lnu{bt}")
                nc.scalar.activation(lnu[:], ut[:], AF.Ln, bias=lnbias[:],
                                     scale=1.0)
                lnt = gum.tile([128, K], F32, tag="lnt", name=f"lnt{bt}")
                nc.scalar.activation(lnt[:], lnu[:], AF.Ln, bias=lnbias[:],
                                     scale=-1.0)
                lnus[bt] = lnu
                lnts[bt] = lnt

            def emit_main(bt):
                g, btm = divmod(bt, 4)
                lnu, lnt = lnus.pop(bt), lnts.pop(bt)

                # MM2: l = 2 h_n @ E^T - e2 via K=2 e2 rows + 6 bf16 passes
                lml = soft.tile([128, K], F32, tag="lml", name=f"lml{bt}")
                for kq in range(NKQ):
                    lp = ps_l.tile([128, 512], F32, tag="l", name=f"l{bt}{kq}")
                    ksl = slice(kq * 512, (kq + 1) * 512)
                    nc.tensor.matmul(
                        lp[:], ones2[:].bitcast(F32R), e2_t[:, ksl],
                        start=True, stop=False,
                    )
                    bsl = slice(bt * 128, (bt + 1) * 128)
                    for dh in range(NDH):
                        nc.tensor.matmul(lp[:], hh[dh][:, bsl],
                                         ETh_t[dh][:, ksl],
                                         start=False, stop=False)
                        nc.tensor.matmul(lp[:], hl[dh][:, bsl],
                                         ETh_t[dh][:, ksl],
                                         start=False, stop=False)
                        nc.tensor.matmul(lp[:], hh[dh][:, bsl],
                                         ETl_t[dh][:, ksl],
                                         start=False, stop=(dh == NDH - 1))
                    # lml = l - ln t  (frees the PSUM chunk immediately)
                    nc.vector.scalar_tensor_tensor(
                        lml[:, ksl], lp[:], 1.0, lnt[:, ksl],
                        op0=OP.bypass, op1=OP.subtract,
                    )

                # m2 = rowmax(lml) stabilizes BOTH softmax numerators
                m2 = vec.tile([128, 1], F32, tag="m2", name=f"m2{bt}")
                nc.vector.tensor_reduce(m2[:], lml[:], axis=AX.X, op=OP.max)
                negm2 = vec.tile([128, 1], F32, tag="negm2",
                                 name=f"negm2{bt}")
                nc.vector.tensor_scalar_mul(negm2[:], m2[:], -1.0)

                # A_g = exp(lml - m2) with S_a row-sum accumulated
                Ag = soft.tile([128, K], BF16, tag="Ag", name=f"Ag{bt}")
                Sa = vec.tile([128, 1], F32, tag="Sa", name=f"Sa{bt}")
                nc.scalar.activation(
                    Ag[:], lml[:], AF.Exp, bias=negm2[:], scale=1.0,
                    accum_out=Sa[:],
                )

                # A = A_g * (-ln u) = exp(l - m2); S_p accumulated
                At = soft.tile([128, K], F32, tag="A", name=f"A{bt}")
                Sp = vec.tile([128, 1], F32, tag="Sp", name=f"Sp{bt}")
                nc.vector.scalar_tensor_tensor(
                    At[:], lnu[:], -1.0, Ag[:],
                    op0=OP.mult, op1=OP.mult, accum_out=Sp[:],
                )
                rSp = vec.tile([128, 1], F32, tag="rSp", name=f"rSp{bt}")
                nc.vector.reciprocal(rSp[:], Sp[:])
                rSa = vec.tile([128, 1], F32R, tag="rSa", name=f"rSa{bt}")
                with nc.allow_low_precision("1/S_a at fp32r is plenty"):
                    nc.vector.reciprocal(rSa[:], Sa[:])

                # probs = A / S_p -> DRAM
                pr = soft.tile([128, K], F32, tag="probs", name=f"pr{bt}")
                nc.vector.tensor_scalar_mul(pr[:], At[:], rSp[:])
                nc.sync.dma_start(probs_d[bt * 128:(bt + 1) * 128, :], pr[:])

                # transpose A_g into AgT (4 code-chunks per PSUM bank)
                for kt0 in range(0, NKT, 4):
                    scr = ps_scr.tile([128, 512], BF16, tag="tscr",
                                      name=f"tscr{bt}{kt0}")
                    for j in range(4):
                        kt = kt0 + j
                        nc.tensor.transpose(
                            scr[:, j * 128:(j + 1) * 128],
                            Ag[:, kt * 128:(kt + 1) * 128],
                            eyeb_t[:],
                        )
                    dst = AgT4[:, kt0:kt0 + 4, btm:btm + 1, :]
                    nc.vector.tensor_copy(dst, scr[:])

                # 1/S_a into the group row (free-dim layout via PE transpose)
                rscr = ps_aux.tile([1, 128], F32, tag="aux",
                                   name=f"rscr{bt}")
                nc.tensor.transpose(rscr[:].bitcast(F32R), rSa[:], eyer_t[:])
                nc.vector.tensor_copy(
                    rs_row[g][:, btm * 128:(btm + 1) * 128], rscr[:]
                )

                # group boundary: MM3  z^T = E^T A_g^T, scaled by 1/S_a
                if btm == 3:
                    bc_ps = ps_aux.tile([128, 512], F32, tag="aux",
                                        name=f"bc{g}")
                    nc.tensor.matmul(
                        bc_ps[:], ones_row[:].bitcast(F32R), rs_row[g][:],
                        start=True, stop=True,
                    )
                    bc_sb = soft.tile([128, 512], F32, tag="bcsb", bufs=1,
                                      name=f"bcsb{g}")
                    nc.vector.tensor_copy(bc_sb[:], bc_ps[:])
                    for dh in range(NDH):
                        zp = ps_z.tile([128, 512], F32, tag=f"z{dh}",
                                       name=f"z{g}{dh}")
                        for kt in range(NKT):
                            nc.tensor.matmul(
                                zp[:],
                                El_t[:, kt * D + dh * 128:
                                     kt * D + dh * 128 + 128],
                                AgT[:, kt * 512:(kt + 1) * 512],
                                start=(kt == 0),
                                stop=(kt == NKT - 1),
                            )
                        zsb = soft.tile([128, 512], F32, tag="zsb",
                                        name=f"zsb{g}{dh}")
                        nc.vector.scalar_tensor_tensor(
                            zsb[:], zp[:], 1.0, bc_sb[:],
                            op0=OP.bypass, op1=OP.mult,
                        )
                        nc.sync.dma_start(
                            zT_d[dh * 128:(dh + 1) * 128,
                                 g * 512:(g + 1) * 512],
                            zsb[:],
                        )

            for pair in range(NBT // 2):
                emit_gumbel(2 * pair)
                emit_gumbel(2 * pair + 1)
                emit_main(2 * pair)
                emit_main(2 * pair + 1)


_nc_cache = None


def _get_program():
    global _nc_cache
    if _nc_cache is None:
        _nc_cache = _build_program()
    return _nc_cache


def _bf16_split(a):
    import ml_dtypes

    hi = a.astype(ml_dtypes.bfloat16)
    lo = (a - hi.astype(np.float32)).astype(ml_dtypes.bfloat16)
    return hi, lo


def kernel(x, u, W, b, gamma, beta, E):
    x = np.asarray(x, dtype=np.float32)
    u = np.asarray(u, dtype=np.float32)
    W = np.asarray(W, dtype=np.float32)
    gamma = np.asarray(gamma, dtype=np.float32)
    beta = np.asarray(beta, dtype=np.float32)
    E = np.asarray(E, dtype=np.float32)

    # device-friendly layouts / operand splits (host-side data prep)
    Wh, Wl = _bf16_split(
        W.reshape(NKC, 128, D).transpose(1, 0, 2).reshape(128, NKC * D)
    )
    El = np.ascontiguousarray(
        E.reshape(NKT, 128, D).transpose(1, 0, 2).reshape(128, NKT * D)
    ).astype(Wh.dtype)
    ETh, ETl = _bf16_split(np.ascontiguousarray(E.T))
    # -|e_k|^2 as (hi, lo) rows; hi keeps 13 mantissa bits so the fp32r
    # matmul path adds it exactly
    e2 = -np.square(E.astype(np.float64)).sum(axis=1)
    e2hi = np.float32(e2)
    e2hi = np.frombuffer(
        (np.frombuffer(e2hi.tobytes(), dtype=np.uint32)
         & np.uint32(0xFFFFE000)).tobytes(),
        dtype=np.float32,
    )
    e2lo = (e2 - e2hi).astype(np.float32)
    e2hl = np.ascontiguousarray(np.stack([e2hi, e2lo]))
    eye = np.eye(128, dtype=np.float32)

    in_maps = []
    for i in range(N_CORES):
        sl = slice(i * BC, (i + 1) * BC)
        xT = np.ascontiguousarray(x[sl].T)
        xh, xl = _bf16_split(xT)
        in_maps.append({
            "xh": xh,
            "xl": xl,
            "u": np.ascontiguousarray(u[sl]),
            "Wh": Wh,
            "Wl": Wl,
            "gamma": np.ascontiguousarray(gamma.reshape(D, 1)),
            "beta": np.ascontiguousarray(beta.reshape(D, 1)),
            "El": El,
            "ETh": ETh,
            "ETl": ETl,
            "e2hl": e2hl,
            "eyeb": eye.astype(Wh.dtype),
            "eyer": eye,
        })

    nc = _get_program()
    trace = os.environ.get("VQ_KERNEL_TRACE", "0") == "1"
    res = run_bass_kernel_spmd(nc, in_maps, list(range(N_CORES)), trace=trace)
    if trace:
        kernel.last_exec_time_ns = res.exec_time_ns

    probs = np.concatenate([res.results[i]["probs"] for i in range(N_CORES)],
                           axis=0)
    z = np.concatenate([res.results[i]["zT"].T for i in range(N_CORES)],
                       axis=0)
    return probs, z


kernel.last_exec_time_ns = None
